# revision 10
# baseline (speedup 1.0000x reference)
"""CRF loss (forward-algorithm log-partition + gold-path score) on 8 Trainium2 cores.

Strategy
--------
Data parallel over the batch (512 -> 64 sequences/core) PLUS sequence-parallel
within each core via a stitched segmented forward algorithm.

The log-partition is Z_b = u^T M_{S-1} ... M_1 f0 with M_s = diag(w_s) E^T,
w_s = exp(emit_s - MU), E = exp(transitions), f0 = exp(st + emit_0 - MU),
u = exp(end).  Products of >=32 positive matrices here are numerically rank-1
(Birkhoff contraction), so the chain splits into K segments whose actions on
arbitrary positive vectors are computed INDEPENDENTLY:
  F_k = M_k b_k   (forward chains,  segments 0..K-2; b_0 = f0, else ones)
  G_k = M_k^T c_k (backward chains, segments 1..K-1; c_{K-1} = u, else ones)
and stitched exactly (error ~ contraction^seg_len ~ 1e-20):
  ln Z = sum_k ln(G_{k+1}^T F_k) - sum_{interior k} ln(1^T F_k) + cnt*MU

With K=16 the sequential depth drops 512 -> 64 rotations.  All 30 chains run
in lockstep: the 15 fwd states live on SBUF partitions 0-63, the 15 bwd
states on partitions 64-127, as one [128, 960] bf16 tensor.  A single
constant block-diagonal stationary blockdiag(E, E^T) makes ONE matmul advance
both directions; one big DVE tensor_tensor applies the (host pre-exp'd, bf16)
emission weights.  Per rotation: 2 matmuls + 2 multiplies (two 480-column
groups for PSUM-bank size and PE/DVE pipelining) instead of the baseline's
per-time-step [64,64] round trips -> the DVE per-instruction overhead is
amortized 7.5x and the critical path is engine throughput, not chain latency.

Numerator (gold-path score) and the final stitch are computed on host.
"""

import sys

import numpy as np

sys.path.insert(0, "/opt/trn_rl_repo")

B, S, T = 512, 1024, 64
NCORES = 8
BPC = B // NCORES  # 64 sequences per core
MU = 4.6559  # calibrated mean log-growth per step of the scaled forward scan

KSEG = 16
NOPS = S - 1  # operator steps 1..S-1
LROT = -(-NOPS // KSEG)  # rotations (sequential depth)
NCH = KSEG - 1  # fwd (=bwd) chain count
FCOLS = NCH * BPC  # state columns per partition half
NGRP = 2
# group 0 takes the ScalarE-copy + 2x-mode-DVE path (PSUM bank caps it at 512
# fp32 columns); group 1 takes the direct 1x PSUM tensor_tensor path
GSPLIT = [512, FCOLS - 512]
GOFF = [0, 512]
# w-chunk sizes along the rotation axis (leading chunks small so the first
# rotation starts as soon as its DMA lands)
_CH = [2, 2, 4, 8, 8]
_CH += [16] * ((LROT - sum(_CH)) // 16)
if sum(_CH) < LROT:
    _CH.append(LROT - sum(_CH))

_BUILD_CACHE = {}


def _segment_sizes():
    """Op-step counts per segment. Segments 0 and K-1 get exactly LROT ops
    (their chains carry true boundary inits and cannot be padded); the
    deficit is taken from interior segments, whose chains start from
    arbitrary positive vectors and absorb a leading identity-emission pad."""
    sizes = [LROT] * KSEG
    deficit = KSEG * LROT - NOPS
    k = 1
    while deficit > 0:
        assert k <= KSEG - 2
        sizes[k] -= 1
        deficit -= 1
        k += 1
    return sizes


def _build_bass():
    import concourse.tile as tile
    from concourse import bacc, mybir

    f32 = mybir.dt.float32
    bf16 = mybir.dt.bfloat16

    nc = bacc.Bacc(None)
    wem = nc.declare_dram_parameter("wem", [128, LROT, FCOLS], bf16, isOutput=False)
    s0 = nc.declare_dram_parameter("s0", [128, FCOLS], bf16, isOutput=False)
    blk = nc.declare_dram_parameter("blk", [128, 128], bf16, isOutput=False)
    fin = nc.declare_dram_parameter("fin", [128, FCOLS], f32, isOutput=True)

    Copy = mybir.ActivationFunctionType.Copy

    with tile.TileContext(nc) as tc:
        with (
            tc.tile_pool(name="const", bufs=1) as const,
            tc.tile_pool(name="state", bufs=1) as state,
            tc.tile_pool(name="mid", bufs=3) as midp,
            tc.tile_pool(name="ps", bufs=4, space="PSUM") as psum,
        ):
            EC = const.tile([128, 128], bf16)
            nc.sync.dma_start(EC[:, :], blk[:, :])

            # initial states FIRST: the first matmul needs them, so their DMA
            # must not queue behind the bulk w transfers
            cur = []
            nxt = []
            finals = []
            for g in range(NGRP):
                gw = GSPLIT[g]
                sa = state.tile([128, gw], bf16, name=f"sa{g}")
                sb = state.tile([128, gw], bf16, name=f"sb{g}")
                sf = state.tile([128, gw], f32, name=f"sf{g}")
                nc.sync.dma_start(sa[:, :], s0[:, GOFF[g] : GOFF[g] + gw])
                cur.append(sa)
                nxt.append(sb)
                finals.append(sf)

            # all w chunks live in SBUF simultaneously (120 KB/partition)
            chunks = []
            r0 = 0
            for i, csz in enumerate(_CH):
                wch = const.tile([128, csz * FCOLS], bf16, name=f"wch{i}")
                wv = wch.rearrange("p (s f) -> p s f", f=FCOLS)
                nc.sync.dma_start(wv, wem[:, r0 : r0 + csz, :])
                chunks.append((r0, csz, wch))
                r0 += csz

            def wslice(r, g):
                for rr0, csz, wch in chunks:
                    if rr0 <= r < rr0 + csz:
                        base = (r - rr0) * FCOLS + GOFF[g]
                        return wch[:, base : base + GSPLIT[g]]
                raise AssertionError

            # the stationary blockdiag(E, E^T) never changes: load it into the
            # PE array once; every matmul skips its self-load
            nc.tensor.ldweights(EC[:, :])

            for r in range(LROT):
                for g in range(NGRP):
                    gw = GSPLIT[g]
                    ps = psum.tile([128, gw], f32, tag="ps")
                    mm = nc.tensor.matmul(
                        ps[:, :], lhsT=EC[:, :], rhs=cur[g][:, :], start=True, stop=True
                    )
                    mm.ins.ldweights = False
                    dst = finals[g] if r == LROT - 1 else nxt[g]
                    if g == 0:
                        # ScalarE evacuates PSUM -> SBUF bf16; DVE multiplies
                        # in 2x packed mode (both operands bf16, SBUF)
                        md = midp.tile([128, gw], bf16, tag="md")
                        nc.scalar.activation(md[:, :], ps[:, :], Copy)
                        nc.vector.tensor_tensor(
                            dst[:, :], md[:, :], wslice(r, g), op=mybir.AluOpType.mult
                        )
                    else:
                        nc.vector.tensor_tensor(
                            dst[:, :], ps[:, :], wslice(r, g), op=mybir.AluOpType.mult
                        )
                cur, nxt = nxt, cur

            for g in range(NGRP):
                nc.sync.dma_start(fin[:, GOFF[g] : GOFF[g] + GSPLIT[g]], finals[g][:, :])
    if not nc.is_finalized():
        nc.finalize()
    return nc


def _get_nc():
    if "nc" not in _BUILD_CACHE:
        _BUILD_CACHE["nc"] = _build_bass()
    return _BUILD_CACHE["nc"]


def _host_numerator(inputs, transitions, start_transitions, end_transitions, tags, mask):
    mf = mask.astype(np.float32)
    score = start_transitions[tags[:, 0]].astype(np.float32)
    trans_score = transitions[tags[:, :-1], tags[:, 1:]]
    emit_score = np.take_along_axis(inputs[:, :-1, :], tags[:, :-1, None], axis=2)[..., 0]
    score = (
        score
        + (trans_score * mf[:, 1:]).sum(1, dtype=np.float32)
        + (emit_score * mf[:, :-1]).sum(1, dtype=np.float32)
    )
    last_idx = mask.astype(np.int32).sum(1) - 1
    last_tags = np.take_along_axis(tags, last_idx[:, None], axis=1)[:, 0]
    last_input = np.take_along_axis(inputs[:, -1, :], last_tags[:, None], axis=1)[:, 0]
    score = score + end_transitions[last_tags] + last_input * mf[:, -1]
    return score  # (B,)


def _host_denominator(inputs, transitions, start_transitions, end_transitions, mask):
    # fallback path (general mask) — numpy mirror of the reference forward algorithm
    alpha = start_transitions[None, :] + inputs[:, 0, :]
    for s in range(1, S):
        inner = alpha[:, :, None] + transitions[None, :, :]
        m = inner.max(axis=1, keepdims=True)
        new = inputs[:, s, :] + np.squeeze(m, 1) + np.log(
            np.exp(inner - m).sum(axis=1)
        )
        alpha = np.where(mask[:, s][:, None], new, alpha)
    stops = alpha + end_transitions[None, :]
    m = stops.max(axis=1, keepdims=True)
    return np.squeeze(m, 1) + np.log(np.exp(stops - m).sum(axis=1))


def _ensure_ntff_hook(bass_utils):
    """Dev-loop only: register the axon NTFF profile hook if the image's
    antenv package lacks axon_hooks (tracing degrades silently otherwise)."""
    import types

    try:
        from antenv.axon_hooks import get_axon_ntff_profile_hook  # noqa: F401

        return
    except ImportError:
        pass
    try:
        import antenv
        from trn_agent_boot.trn_boot import _ntff_profile_via_ctypes

        mod = types.ModuleType("antenv.axon_hooks")
        holder = {"h": None}
        mod.set_axon_ntff_profile_hook = lambda h: holder.__setitem__("h", h)
        mod.get_axon_ntff_profile_hook = lambda: holder["h"]
        sys.modules["antenv.axon_hooks"] = mod
        antenv.axon_hooks = mod
        hook = _ntff_profile_via_ctypes("/opt/axon/libaxon_pjrt.so")
        if hook is not None:
            mod.set_axon_ntff_profile_hook(hook)
        # zero-egress container: skip the artifact upload in the trace path
        bass_utils.upload_artifacts = lambda tmpdir: tmpdir
    except Exception as e:  # pragma: no cover
        print("ntff hook setup failed:", e)


def _prep_core(xt, st, en, sizes, bounds, bf16):
    """Build the device inputs for one core.
    xt: [T(tag), S, BPC] f32 emissions for this core's sequences."""
    w = np.exp(xt - np.float32(MU), dtype=np.float32)  # [T, S, b]

    wem = np.empty((128, LROT, FCOLS), dtype=bf16)
    s0 = np.empty((128, FCOLS), dtype=bf16)
    for c in range(NCH):
        cols = slice(c * BPC, (c + 1) * BPC)
        # fwd chain for segment c (partitions 0..T-1)
        a, b_ = bounds[c]
        pad = LROT - sizes[c]
        if pad:
            wem[:T, :pad, cols] = np.float32(1.0)
        wem[:T, pad:, cols] = w[:, a:b_, :]
        if c == 0:
            s0[:T, cols] = np.exp(
                st[:, None] + xt[:, 0, :] - np.float32(MU), dtype=np.float32
            )
        else:
            s0[:T, cols] = np.float32(1.0)
        # bwd chain for segment c+1 (partitions T..2T-1)
        k = c + 1
        a, b_ = bounds[k]
        pad = LROT - sizes[k]
        if pad == 0:
            init = w[:, b_ - 1, :]
            if k == KSEG - 1:
                init = init * np.exp(en[:, None])
            s0[T:, cols] = init
            wem[T:, : sizes[k] - 1, cols] = w[:, b_ - 2 : a - 1 if a else None : -1, :]
            wem[T:, sizes[k] - 1 :, cols] = np.float32(1.0)
        else:
            assert pad == 1 and k != KSEG - 1
            s0[T:, cols] = np.float32(1.0)
            wem[T:, : sizes[k], cols] = w[:, b_ - 1 : a - 1 if a else None : -1, :]
            wem[T:, sizes[k] :, cols] = np.float32(1.0)
    return wem, s0


def _stitch(fin, sizes, E):
    """fin: [128, FCOLS] f32 device output -> per-sequence logZ [BPC] (f64)."""
    f = fin.astype(np.float64)
    top = f[:T].reshape(T, NCH, BPC)  # F_k for k=0..K-2
    bot = f[T:].reshape(T, NCH, BPC)  # G_{k+1} at block k
    d = (top * bot).sum(axis=0)  # [NCH, BPC]
    logZ = np.log(d).sum(axis=0)
    nF = top.sum(axis=0)  # [NCH, BPC]; interior F_k are k=1..K-2 -> blocks 1..NCH-1
    logZ -= np.log(nF[1:]).sum(axis=0)
    cntF = [sizes[k] + (1 if k == 0 else 0) for k in range(KSEG)]
    cntG = list(sizes)
    cnt = sum(cntF[k] + cntG[k + 1] for k in range(KSEG - 1)) - sum(
        cntF[k] for k in range(1, KSEG - 1)
    )
    # padded interior segments: the bwd chain's leading identity-emission pad
    # applies E before its arbitrary init, making its effective init E^pad @ 1
    # whose projection onto the segment's Perron direction (== F_k direction)
    # does not cancel in the stitch; remove it exactly.
    E64 = E.astype(np.float64)
    for k in range(1, KSEG - 1):
        pad = LROT - sizes[k]
        if pad:
            v = np.linalg.matrix_power(E64, pad) @ np.ones(T)
            Fk = top[:, k, :]
            logZ -= np.log(Fk.T @ v) - np.log(Fk.sum(axis=0))
    return logZ + cnt * MU


def kernel(inputs, transitions, start_transitions, end_transitions, tags, mask):
    inputs = np.ascontiguousarray(np.asarray(inputs), dtype=np.float32)
    transitions = np.asarray(transitions, dtype=np.float32)
    start_transitions = np.asarray(start_transitions, dtype=np.float32)
    end_transitions = np.asarray(end_transitions, dtype=np.float32)
    tags = np.asarray(tags)
    mask_b = np.asarray(mask).astype(bool)

    num = _host_numerator(
        inputs, transitions, start_transitions, end_transitions, tags.astype(np.int64), mask_b
    )

    if not mask_b.all():
        den = _host_denominator(
            inputs.astype(np.float64),
            transitions.astype(np.float64),
            start_transitions.astype(np.float64),
            end_transitions.astype(np.float64),
            mask_b,
        ).astype(np.float32)
        return np.asarray(
            np.float32(num.sum(dtype=np.float32)) - np.float32(den.sum(dtype=np.float32)),
            dtype=np.float32,
        )

    import os

    from concourse import bass_utils

    trace = bool(int(os.environ.get("CRF_TRACE", "0")))
    if trace:
        _ensure_ntff_hook(bass_utils)


    import ml_dtypes

    bf16 = ml_dtypes.bfloat16

    sizes = _segment_sizes()
    bounds = []
    a = 1
    for sz in sizes:
        bounds.append((a, a + sz))
        a += sz
    assert bounds[-1][1] == S

    E = np.exp(transitions).astype(bf16).astype(np.float32)
    blk = np.zeros((128, 128), dtype=bf16)
    blk[:T, :T] = E  # lhsT for fwd: out[j] = sum_i E[i,j] F[i]
    blk[T:, T:] = np.ascontiguousarray(E.T)  # lhsT for bwd: out[j] = sum_i E[j,i] G[i]

    nc = _get_nc()
    in_maps = []
    for c in range(NCORES):
        shard = inputs[c * BPC : (c + 1) * BPC]  # [b, s, j]
        xt = np.ascontiguousarray(shard.transpose(2, 1, 0))  # [j, s, b]
        wem, s0 = _prep_core(xt, start_transitions, end_transitions, sizes, bounds, bf16)
        in_maps.append({"wem": wem, "s0": s0, "blk": blk})

    res = bass_utils.run_bass_kernel_spmd(
        nc, in_maps, core_ids=list(range(NCORES)), trace=trace
    )
    if trace and res.exec_time_ns is not None:
        print(f"HW exec time: {res.exec_time_ns} ns")
        if res.instructions_and_trace is not None:
            print("trace:", res.instructions_and_trace[1])

    logZ = np.concatenate([_stitch(r["fin"], sizes, E) for r in res.results])
    loss = np.float64(num.sum(dtype=np.float64)) - logZ.sum()
    return np.asarray(loss, dtype=np.float32)


# revision 11
# speedup vs baseline: 1.4034x; 1.4034x over previous
"""CRF loss (forward-algorithm log-partition + gold-path score) on 8 Trainium2 cores.

Strategy
--------
Data parallel over the batch (512 -> 64 sequences/core) PLUS sequence-parallel
within each core via a stitched segmented forward algorithm.

The log-partition is Z_b = u^T M_{S-1} ... M_1 f0 with M_s = diag(w_s) E^T,
w_s = exp(emit_s - MU), E = exp(transitions), f0 = exp(st + emit_0 - MU),
u = exp(end).  Products of >=32 positive matrices here are numerically rank-1
(Birkhoff contraction), so the chain splits into K segments whose actions on
arbitrary positive vectors are computed INDEPENDENTLY:
  F_k = M_k b_k   (forward chains,  segments 0..K-2; b_0 = f0, else ones)
  G_k = M_k^T c_k (backward chains, segments 1..K-1; c_{K-1} = u, else ones)
and stitched exactly (error ~ contraction^seg_len ~ 1e-20):
  ln Z = sum_k ln(G_{k+1}^T F_k) - sum_{interior k} ln(1^T F_k) + cnt*MU

With K=16 the sequential depth drops 512 -> 64 rotations.  All 30 chains run
in lockstep: the 15 fwd states live on SBUF partitions 0-63, the 15 bwd
states on partitions 64-127, as one [128, 960] bf16 tensor.  A single
constant block-diagonal stationary blockdiag(E, E^T) makes ONE matmul advance
both directions; one big DVE tensor_tensor applies the (host pre-exp'd, bf16)
emission weights.  Per rotation: 2 matmuls + 2 multiplies (two 480-column
groups for PSUM-bank size and PE/DVE pipelining) instead of the baseline's
per-time-step [64,64] round trips -> the DVE per-instruction overhead is
amortized 7.5x and the critical path is engine throughput, not chain latency.

Numerator (gold-path score) and the final stitch are computed on host.
"""

import sys

import numpy as np

sys.path.insert(0, "/opt/trn_rl_repo")

B, S, T = 512, 1024, 64
NCORES = 8
BPC = B // NCORES  # 64 sequences per core
MU = 4.6559  # calibrated mean log-growth per step of the scaled forward scan

KSEG = 16
NOPS = S - 1  # operator steps 1..S-1
LROT = -(-NOPS // KSEG)  # rotations (sequential depth)
NCH = KSEG - 1  # fwd (=bwd) chain count
FCOLS = NCH * BPC  # state columns per partition half
NGRP = 2
# group 0 takes the ScalarE-copy + 2x-mode-DVE path (PSUM bank caps it at 512
# fp32 columns); group 1 takes the direct 1x PSUM tensor_tensor path
GSPLIT = [512, FCOLS - 512]
GOFF = [0, 512]
# w-chunk sizes along the rotation axis (leading chunks small so the first
# rotation starts as soon as its DMA lands)
_CH = [2, 2, 4, 8, 8]
_CH += [16] * ((LROT - sum(_CH)) // 16)
if sum(_CH) < LROT:
    _CH.append(LROT - sum(_CH))

_BUILD_CACHE = {}


def _segment_sizes():
    """Op-step counts per segment. Segments 0 and K-1 get exactly LROT ops
    (their chains carry true boundary inits and cannot be padded); the
    deficit is taken from interior segments, whose chains start from
    arbitrary positive vectors and absorb a leading identity-emission pad."""
    sizes = [LROT] * KSEG
    deficit = KSEG * LROT - NOPS
    k = 1
    while deficit > 0:
        assert k <= KSEG - 2
        sizes[k] -= 1
        deficit -= 1
        k += 1
    return sizes


def _build_bass():
    import concourse.tile as tile
    from concourse import bacc, mybir

    f32 = mybir.dt.float32
    bf16 = mybir.dt.bfloat16

    nc = bacc.Bacc(None)
    wem = nc.declare_dram_parameter("wem", [128, LROT, FCOLS], bf16, isOutput=False)
    s0 = nc.declare_dram_parameter("s0", [128, FCOLS], bf16, isOutput=False)
    blk = nc.declare_dram_parameter("blk", [128, 128], bf16, isOutput=False)
    fin = nc.declare_dram_parameter("fin", [128, FCOLS], f32, isOutput=True)

    Copy = mybir.ActivationFunctionType.Copy

    with tile.TileContext(nc) as tc:
        with (
            tc.tile_pool(name="const", bufs=1) as const,
            tc.tile_pool(name="state", bufs=1) as state,
            tc.tile_pool(name="mid", bufs=3) as midp,
            tc.tile_pool(name="ps", bufs=4, space="PSUM") as psum,
        ):
            EC = const.tile([128, 128], bf16)
            nc.sync.dma_start(EC[:, :], blk[:, :])

            # initial states FIRST: the first matmul needs them, so their DMA
            # must not queue behind the bulk w transfers
            cur = []
            nxt = []
            finals = []
            for g in range(NGRP):
                gw = GSPLIT[g]
                sa = state.tile([128, gw], bf16, name=f"sa{g}")
                sb = state.tile([128, gw], bf16, name=f"sb{g}")
                sf = state.tile([128, gw], f32, name=f"sf{g}")
                nc.sync.dma_start(sa[:, :], s0[:, GOFF[g] : GOFF[g] + gw])
                cur.append(sa)
                nxt.append(sb)
                finals.append(sf)

            # all w chunks live in SBUF simultaneously (120 KB/partition)
            chunks = []
            r0 = 0
            for i, csz in enumerate(_CH):
                wch = const.tile([128, csz * FCOLS], bf16, name=f"wch{i}")
                wv = wch.rearrange("p (s f) -> p s f", f=FCOLS)
                nc.sync.dma_start(wv, wem[:, r0 : r0 + csz, :])
                chunks.append((r0, csz, wch))
                r0 += csz

            def wslice(r, g):
                for rr0, csz, wch in chunks:
                    if rr0 <= r < rr0 + csz:
                        base = (r - rr0) * FCOLS + GOFF[g]
                        return wch[:, base : base + GSPLIT[g]]
                raise AssertionError

            # the stationary blockdiag(E, E^T) never changes: load it into the
            # PE array once; every matmul skips its self-load
            nc.tensor.ldweights(EC[:, :])

            for r in range(LROT):
                for g in range(NGRP):
                    gw = GSPLIT[g]
                    ps = psum.tile([128, gw], f32, tag="ps")
                    mm = nc.tensor.matmul(
                        ps[:, :], lhsT=EC[:, :], rhs=cur[g][:, :], start=True, stop=True
                    )
                    mm.ins.ldweights = False
                    dst = finals[g] if r == LROT - 1 else nxt[g]
                    nc.vector.tensor_tensor(
                        dst[:, :], ps[:, :], wslice(r, g), op=mybir.AluOpType.mult
                    )
                cur, nxt = nxt, cur

            for g in range(NGRP):
                nc.sync.dma_start(fin[:, GOFF[g] : GOFF[g] + GSPLIT[g]], finals[g][:, :])
    if not nc.is_finalized():
        nc.finalize()
    return nc


def _get_nc():
    if "nc" not in _BUILD_CACHE:
        _BUILD_CACHE["nc"] = _build_bass()
    return _BUILD_CACHE["nc"]


def _host_numerator(inputs, transitions, start_transitions, end_transitions, tags, mask):
    mf = mask.astype(np.float32)
    score = start_transitions[tags[:, 0]].astype(np.float32)
    trans_score = transitions[tags[:, :-1], tags[:, 1:]]
    emit_score = np.take_along_axis(inputs[:, :-1, :], tags[:, :-1, None], axis=2)[..., 0]
    score = (
        score
        + (trans_score * mf[:, 1:]).sum(1, dtype=np.float32)
        + (emit_score * mf[:, :-1]).sum(1, dtype=np.float32)
    )
    last_idx = mask.astype(np.int32).sum(1) - 1
    last_tags = np.take_along_axis(tags, last_idx[:, None], axis=1)[:, 0]
    last_input = np.take_along_axis(inputs[:, -1, :], last_tags[:, None], axis=1)[:, 0]
    score = score + end_transitions[last_tags] + last_input * mf[:, -1]
    return score  # (B,)


def _host_denominator(inputs, transitions, start_transitions, end_transitions, mask):
    # fallback path (general mask) — numpy mirror of the reference forward algorithm
    alpha = start_transitions[None, :] + inputs[:, 0, :]
    for s in range(1, S):
        inner = alpha[:, :, None] + transitions[None, :, :]
        m = inner.max(axis=1, keepdims=True)
        new = inputs[:, s, :] + np.squeeze(m, 1) + np.log(
            np.exp(inner - m).sum(axis=1)
        )
        alpha = np.where(mask[:, s][:, None], new, alpha)
    stops = alpha + end_transitions[None, :]
    m = stops.max(axis=1, keepdims=True)
    return np.squeeze(m, 1) + np.log(np.exp(stops - m).sum(axis=1))


def _ensure_ntff_hook(bass_utils):
    """Dev-loop only: register the axon NTFF profile hook if the image's
    antenv package lacks axon_hooks (tracing degrades silently otherwise)."""
    import types

    try:
        from antenv.axon_hooks import get_axon_ntff_profile_hook  # noqa: F401

        return
    except ImportError:
        pass
    try:
        import antenv
        from trn_agent_boot.trn_boot import _ntff_profile_via_ctypes

        mod = types.ModuleType("antenv.axon_hooks")
        holder = {"h": None}
        mod.set_axon_ntff_profile_hook = lambda h: holder.__setitem__("h", h)
        mod.get_axon_ntff_profile_hook = lambda: holder["h"]
        sys.modules["antenv.axon_hooks"] = mod
        antenv.axon_hooks = mod
        hook = _ntff_profile_via_ctypes("/opt/axon/libaxon_pjrt.so")
        if hook is not None:
            mod.set_axon_ntff_profile_hook(hook)
        # zero-egress container: skip the artifact upload in the trace path
        bass_utils.upload_artifacts = lambda tmpdir: tmpdir
    except Exception as e:  # pragma: no cover
        print("ntff hook setup failed:", e)


def _prep_core(xt, st, en, sizes, bounds, bf16):
    """Build the device inputs for one core.
    xt: [T(tag), S, BPC] f32 emissions for this core's sequences."""
    w = np.exp(xt - np.float32(MU), dtype=np.float32)  # [T, S, b]

    wem = np.empty((128, LROT, FCOLS), dtype=bf16)
    s0 = np.empty((128, FCOLS), dtype=bf16)
    for c in range(NCH):
        cols = slice(c * BPC, (c + 1) * BPC)
        # fwd chain for segment c (partitions 0..T-1)
        a, b_ = bounds[c]
        pad = LROT - sizes[c]
        if pad:
            wem[:T, :pad, cols] = np.float32(1.0)
        wem[:T, pad:, cols] = w[:, a:b_, :]
        if c == 0:
            s0[:T, cols] = np.exp(
                st[:, None] + xt[:, 0, :] - np.float32(MU), dtype=np.float32
            )
        else:
            s0[:T, cols] = np.float32(1.0)
        # bwd chain for segment c+1 (partitions T..2T-1)
        k = c + 1
        a, b_ = bounds[k]
        pad = LROT - sizes[k]
        if pad == 0:
            init = w[:, b_ - 1, :]
            if k == KSEG - 1:
                init = init * np.exp(en[:, None])
            s0[T:, cols] = init
            wem[T:, : sizes[k] - 1, cols] = w[:, b_ - 2 : a - 1 if a else None : -1, :]
            wem[T:, sizes[k] - 1 :, cols] = np.float32(1.0)
        else:
            assert pad == 1 and k != KSEG - 1
            s0[T:, cols] = np.float32(1.0)
            wem[T:, : sizes[k], cols] = w[:, b_ - 1 : a - 1 if a else None : -1, :]
            wem[T:, sizes[k] :, cols] = np.float32(1.0)
    return wem, s0


def _stitch(fin, sizes, E):
    """fin: [128, FCOLS] f32 device output -> per-sequence logZ [BPC] (f64)."""
    f = fin.astype(np.float64)
    top = f[:T].reshape(T, NCH, BPC)  # F_k for k=0..K-2
    bot = f[T:].reshape(T, NCH, BPC)  # G_{k+1} at block k
    d = (top * bot).sum(axis=0)  # [NCH, BPC]
    logZ = np.log(d).sum(axis=0)
    nF = top.sum(axis=0)  # [NCH, BPC]; interior F_k are k=1..K-2 -> blocks 1..NCH-1
    logZ -= np.log(nF[1:]).sum(axis=0)
    cntF = [sizes[k] + (1 if k == 0 else 0) for k in range(KSEG)]
    cntG = list(sizes)
    cnt = sum(cntF[k] + cntG[k + 1] for k in range(KSEG - 1)) - sum(
        cntF[k] for k in range(1, KSEG - 1)
    )
    # padded interior segments: the bwd chain's leading identity-emission pad
    # applies E before its arbitrary init, making its effective init E^pad @ 1
    # whose projection onto the segment's Perron direction (== F_k direction)
    # does not cancel in the stitch; remove it exactly.
    E64 = E.astype(np.float64)
    for k in range(1, KSEG - 1):
        pad = LROT - sizes[k]
        if pad:
            v = np.linalg.matrix_power(E64, pad) @ np.ones(T)
            Fk = top[:, k, :]
            logZ -= np.log(Fk.T @ v) - np.log(Fk.sum(axis=0))
    return logZ + cnt * MU


def kernel(inputs, transitions, start_transitions, end_transitions, tags, mask):
    inputs = np.ascontiguousarray(np.asarray(inputs), dtype=np.float32)
    transitions = np.asarray(transitions, dtype=np.float32)
    start_transitions = np.asarray(start_transitions, dtype=np.float32)
    end_transitions = np.asarray(end_transitions, dtype=np.float32)
    tags = np.asarray(tags)
    mask_b = np.asarray(mask).astype(bool)

    num = _host_numerator(
        inputs, transitions, start_transitions, end_transitions, tags.astype(np.int64), mask_b
    )

    if not mask_b.all():
        den = _host_denominator(
            inputs.astype(np.float64),
            transitions.astype(np.float64),
            start_transitions.astype(np.float64),
            end_transitions.astype(np.float64),
            mask_b,
        ).astype(np.float32)
        return np.asarray(
            np.float32(num.sum(dtype=np.float32)) - np.float32(den.sum(dtype=np.float32)),
            dtype=np.float32,
        )

    import os

    from concourse import bass_utils

    trace = bool(int(os.environ.get("CRF_TRACE", "0")))
    if trace:
        _ensure_ntff_hook(bass_utils)


    import ml_dtypes

    bf16 = ml_dtypes.bfloat16

    sizes = _segment_sizes()
    bounds = []
    a = 1
    for sz in sizes:
        bounds.append((a, a + sz))
        a += sz
    assert bounds[-1][1] == S

    E = np.exp(transitions).astype(bf16).astype(np.float32)
    blk = np.zeros((128, 128), dtype=bf16)
    blk[:T, :T] = E  # lhsT for fwd: out[j] = sum_i E[i,j] F[i]
    blk[T:, T:] = np.ascontiguousarray(E.T)  # lhsT for bwd: out[j] = sum_i E[j,i] G[i]

    nc = _get_nc()
    in_maps = []
    for c in range(NCORES):
        shard = inputs[c * BPC : (c + 1) * BPC]  # [b, s, j]
        xt = np.ascontiguousarray(shard.transpose(2, 1, 0))  # [j, s, b]
        wem, s0 = _prep_core(xt, start_transitions, end_transitions, sizes, bounds, bf16)
        in_maps.append({"wem": wem, "s0": s0, "blk": blk})

    res = bass_utils.run_bass_kernel_spmd(
        nc, in_maps, core_ids=list(range(NCORES)), trace=trace
    )
    if trace and res.exec_time_ns is not None:
        print(f"HW exec time: {res.exec_time_ns} ns")
        if res.instructions_and_trace is not None:
            print("trace:", res.instructions_and_trace[1])

    logZ = np.concatenate([_stitch(r["fin"], sizes, E) for r in res.results])
    loss = np.float64(num.sum(dtype=np.float64)) - logZ.sum()
    return np.asarray(loss, dtype=np.float32)


# revision 13
# speedup vs baseline: 1.8031x; 1.2849x over previous
"""CRF loss (forward-algorithm log-partition + gold-path score) on 8 Trainium2 cores.

Strategy
--------
Data parallel over the batch (512 -> 64 sequences/core) PLUS sequence-parallel
within each core via a stitched segmented forward algorithm.

The log-partition is Z_b = u^T M_{S-1} ... M_1 f0 with M_s = diag(w_s) E^T,
w_s = exp(emit_s - MU), E = exp(transitions), f0 = exp(st + emit_0 - MU),
u = exp(end).  Products of >=32 positive matrices here are numerically rank-1
(Birkhoff contraction), so the chain splits into K segments whose actions on
arbitrary positive vectors are computed INDEPENDENTLY:
  F_k = M_k b_k   (forward chains,  segments 0..K-2; b_0 = f0, else ones)
  G_k = M_k^T c_k (backward chains, segments 1..K-1; c_{K-1} = u, else ones)
and stitched exactly (error ~ contraction^seg_len ~ 1e-20):
  ln Z = sum_k ln(G_{k+1}^T F_k) - sum_{interior k} ln(1^T F_k) + cnt*MU

With K=16 the sequential depth drops 512 -> 64 rotations.  All 30 chains run
in lockstep: the 15 fwd states live on SBUF partitions 0-63, the 15 bwd
states on partitions 64-127, as one [128, 960] bf16 tensor.  A single
constant block-diagonal stationary blockdiag(E, E^T) makes ONE matmul advance
both directions; one big DVE tensor_tensor applies the (host pre-exp'd, bf16)
emission weights.  Per rotation: 2 matmuls + 2 multiplies (two 480-column
groups for PSUM-bank size and PE/DVE pipelining) instead of the baseline's
per-time-step [64,64] round trips -> the DVE per-instruction overhead is
amortized 7.5x and the critical path is engine throughput, not chain latency.

Numerator (gold-path score) and the final stitch are computed on host.
"""

import sys

import numpy as np

sys.path.insert(0, "/opt/trn_rl_repo")

B, S, T = 512, 1024, 64
NCORES = 8
BPC = B // NCORES  # 64 sequences per core
MU = 4.6559  # calibrated mean log-growth per step of the scaled forward scan

KSEG = 65
NOPS = S - 1  # operator steps 1..S-1
LROT = -(-NOPS // KSEG)  # rotations (sequential depth)
NCH = KSEG - 1  # fwd (=bwd) chain count
FCOLS = NCH * BPC  # state columns per partition half
NGRP = FCOLS // 512  # one PSUM bank (512 fp32 cols) per group
GSPLIT = [512] * NGRP
GOFF = [512 * g for g in range(NGRP)]
# groups 0..NDIRECT-1: direct 1x PSUM tensor_tensor on DVE;
# groups NDIRECT..: ScalarE evacuates PSUM->SBUF bf16, DVE multiplies in 2x
# packed mode -- splits the multiply load across both PSUM-capable engines
NDIRECT = 3
# w-chunk sizes along the rotation axis (leading chunks small so the first
# rotation starts as soon as its DMA lands)
_CH = [1, 1, 2, 2, 2, 4]
if sum(_CH) < LROT:
    _CH.append(LROT - sum(_CH))

_BUILD_CACHE = {}


def _segment_sizes():
    """Op-step counts per segment. Segments 0 and K-1 get exactly LROT ops
    (their chains carry true boundary inits and cannot be padded); the
    deficit is taken from interior segments, whose chains start from
    arbitrary positive vectors and absorb a leading identity-emission pad."""
    sizes = [LROT] * KSEG
    deficit = KSEG * LROT - NOPS
    k = 1
    while deficit > 0:
        assert k <= KSEG - 2
        sizes[k] -= 1
        deficit -= 1
        k += 1
    return sizes


def _build_bass():
    import concourse.tile as tile
    from concourse import bacc, mybir

    f32 = mybir.dt.float32
    bf16 = mybir.dt.bfloat16

    nc = bacc.Bacc(None)
    wem = nc.declare_dram_parameter("wem", [128, LROT, FCOLS], bf16, isOutput=False)
    s0 = nc.declare_dram_parameter("s0", [128, FCOLS], bf16, isOutput=False)
    blk = nc.declare_dram_parameter("blk", [128, 128], bf16, isOutput=False)
    fin = nc.declare_dram_parameter("fin", [128, FCOLS], f32, isOutput=True)

    Copy = mybir.ActivationFunctionType.Copy

    with tile.TileContext(nc) as tc:
        with (
            tc.tile_pool(name="const", bufs=1) as const,
            tc.tile_pool(name="state", bufs=1) as state,
            tc.tile_pool(name="mid", bufs=3) as midp,
            tc.tile_pool(name="ps", bufs=4, space="PSUM") as psum,
        ):
            EC = const.tile([128, 128], bf16)
            nc.sync.dma_start(EC[:, :], blk[:, :])

            # initial states FIRST: the first matmul needs them, so their DMA
            # must not queue behind the bulk w transfers
            cur = []
            nxt = []
            finals = []
            for g in range(NGRP):
                gw = GSPLIT[g]
                sa = state.tile([128, gw], bf16, name=f"sa{g}")
                sb = state.tile([128, gw], bf16, name=f"sb{g}")
                sf = state.tile([128, gw], f32, name=f"sf{g}")
                nc.sync.dma_start(sa[:, :], s0[:, GOFF[g] : GOFF[g] + gw])
                cur.append(sa)
                nxt.append(sb)
                finals.append(sf)

            # all w chunks live in SBUF simultaneously (120 KB/partition)
            chunks = []
            r0 = 0
            for i, csz in enumerate(_CH):
                wch = const.tile([128, csz * FCOLS], bf16, name=f"wch{i}")
                wv = wch.rearrange("p (s f) -> p s f", f=FCOLS)
                nc.sync.dma_start(wv, wem[:, r0 : r0 + csz, :])
                chunks.append((r0, csz, wch))
                r0 += csz

            def wslice(r, g):
                for rr0, csz, wch in chunks:
                    if rr0 <= r < rr0 + csz:
                        base = (r - rr0) * FCOLS + GOFF[g]
                        return wch[:, base : base + GSPLIT[g]]
                raise AssertionError

            # the stationary blockdiag(E, E^T) never changes: load it into the
            # PE array once; every matmul skips its self-load
            nc.tensor.ldweights(EC[:, :])

            for r in range(LROT):
                for g in range(NGRP):
                    gw = GSPLIT[g]
                    ps = psum.tile([128, gw], f32, tag=f"ps{g}", bufs=1)
                    mm = nc.tensor.matmul(
                        ps[:, :], lhsT=EC[:, :], rhs=cur[g][:, :], start=True, stop=True
                    )
                    mm.ins.ldweights = False
                    dst = finals[g] if r == LROT - 1 else nxt[g]
                    if g >= NDIRECT:
                        md = midp.tile([128, gw], bf16, tag=f"md{g}", bufs=2)
                        nc.scalar.activation(md[:, :], ps[:, :], Copy)
                        nc.vector.tensor_tensor(
                            dst[:, :], md[:, :], wslice(r, g), op=mybir.AluOpType.mult
                        )
                    else:
                        nc.vector.tensor_tensor(
                            dst[:, :], ps[:, :], wslice(r, g), op=mybir.AluOpType.mult
                        )
                cur, nxt = nxt, cur

            for g in range(NGRP):
                nc.sync.dma_start(fin[:, GOFF[g] : GOFF[g] + GSPLIT[g]], finals[g][:, :])
    if not nc.is_finalized():
        nc.finalize()
    return nc


def _get_nc():
    if "nc" not in _BUILD_CACHE:
        _BUILD_CACHE["nc"] = _build_bass()
    return _BUILD_CACHE["nc"]


def _host_numerator(inputs, transitions, start_transitions, end_transitions, tags, mask):
    mf = mask.astype(np.float32)
    score = start_transitions[tags[:, 0]].astype(np.float32)
    trans_score = transitions[tags[:, :-1], tags[:, 1:]]
    emit_score = np.take_along_axis(inputs[:, :-1, :], tags[:, :-1, None], axis=2)[..., 0]
    score = (
        score
        + (trans_score * mf[:, 1:]).sum(1, dtype=np.float32)
        + (emit_score * mf[:, :-1]).sum(1, dtype=np.float32)
    )
    last_idx = mask.astype(np.int32).sum(1) - 1
    last_tags = np.take_along_axis(tags, last_idx[:, None], axis=1)[:, 0]
    last_input = np.take_along_axis(inputs[:, -1, :], last_tags[:, None], axis=1)[:, 0]
    score = score + end_transitions[last_tags] + last_input * mf[:, -1]
    return score  # (B,)


def _host_denominator(inputs, transitions, start_transitions, end_transitions, mask):
    # fallback path (general mask) — numpy mirror of the reference forward algorithm
    alpha = start_transitions[None, :] + inputs[:, 0, :]
    for s in range(1, S):
        inner = alpha[:, :, None] + transitions[None, :, :]
        m = inner.max(axis=1, keepdims=True)
        new = inputs[:, s, :] + np.squeeze(m, 1) + np.log(
            np.exp(inner - m).sum(axis=1)
        )
        alpha = np.where(mask[:, s][:, None], new, alpha)
    stops = alpha + end_transitions[None, :]
    m = stops.max(axis=1, keepdims=True)
    return np.squeeze(m, 1) + np.log(np.exp(stops - m).sum(axis=1))


def _ensure_ntff_hook(bass_utils):
    """Dev-loop only: register the axon NTFF profile hook if the image's
    antenv package lacks axon_hooks (tracing degrades silently otherwise)."""
    import types

    try:
        from antenv.axon_hooks import get_axon_ntff_profile_hook  # noqa: F401

        return
    except ImportError:
        pass
    try:
        import antenv
        from trn_agent_boot.trn_boot import _ntff_profile_via_ctypes

        mod = types.ModuleType("antenv.axon_hooks")
        holder = {"h": None}
        mod.set_axon_ntff_profile_hook = lambda h: holder.__setitem__("h", h)
        mod.get_axon_ntff_profile_hook = lambda: holder["h"]
        sys.modules["antenv.axon_hooks"] = mod
        antenv.axon_hooks = mod
        hook = _ntff_profile_via_ctypes("/opt/axon/libaxon_pjrt.so")
        if hook is not None:
            mod.set_axon_ntff_profile_hook(hook)
        # zero-egress container: skip the artifact upload in the trace path
        bass_utils.upload_artifacts = lambda tmpdir: tmpdir
    except Exception as e:  # pragma: no cover
        print("ntff hook setup failed:", e)


def _prep_core(xt, st, en, sizes, bounds, bf16):
    """Build the device inputs for one core.
    xt: [T(tag), S, BPC] f32 emissions for this core's sequences."""
    w = np.exp(xt - np.float32(MU), dtype=np.float32)  # [T, S, b]

    wem = np.empty((128, LROT, FCOLS), dtype=bf16)
    s0 = np.empty((128, FCOLS), dtype=bf16)
    for c in range(NCH):
        cols = slice(c * BPC, (c + 1) * BPC)
        # fwd chain for segment c (partitions 0..T-1)
        a, b_ = bounds[c]
        pad = LROT - sizes[c]
        if pad:
            wem[:T, :pad, cols] = np.float32(1.0)
        wem[:T, pad:, cols] = w[:, a:b_, :]
        if c == 0:
            s0[:T, cols] = np.exp(
                st[:, None] + xt[:, 0, :] - np.float32(MU), dtype=np.float32
            )
        else:
            s0[:T, cols] = np.float32(1.0)
        # bwd chain for segment c+1 (partitions T..2T-1)
        k = c + 1
        a, b_ = bounds[k]
        pad = LROT - sizes[k]
        if pad == 0:
            init = w[:, b_ - 1, :]
            if k == KSEG - 1:
                init = init * np.exp(en[:, None])
            s0[T:, cols] = init
            wem[T:, : sizes[k] - 1, cols] = w[:, b_ - 2 : a - 1 if a else None : -1, :]
            wem[T:, sizes[k] - 1 :, cols] = np.float32(1.0)
        else:
            assert pad == 1 and k != KSEG - 1
            s0[T:, cols] = np.float32(1.0)
            wem[T:, : sizes[k], cols] = w[:, b_ - 1 : a - 1 if a else None : -1, :]
            wem[T:, sizes[k] :, cols] = np.float32(1.0)
    return wem, s0


def _stitch(fin, sizes, E):
    """fin: [128, FCOLS] f32 device output -> per-sequence logZ [BPC] (f64)."""
    f = fin.astype(np.float64)
    top = f[:T].reshape(T, NCH, BPC)  # F_k for k=0..K-2
    bot = f[T:].reshape(T, NCH, BPC)  # G_{k+1} at block k
    d = (top * bot).sum(axis=0)  # [NCH, BPC]
    logZ = np.log(d).sum(axis=0)
    nF = top.sum(axis=0)  # [NCH, BPC]; interior F_k are k=1..K-2 -> blocks 1..NCH-1
    logZ -= np.log(nF[1:]).sum(axis=0)
    cntF = [sizes[k] + (1 if k == 0 else 0) for k in range(KSEG)]
    cntG = list(sizes)
    cnt = sum(cntF[k] + cntG[k + 1] for k in range(KSEG - 1)) - sum(
        cntF[k] for k in range(1, KSEG - 1)
    )
    # padded interior segments: the bwd chain's leading identity-emission pad
    # applies E before its arbitrary init, making its effective init E^pad @ 1
    # whose projection onto the segment's Perron direction (== F_k direction)
    # does not cancel in the stitch; remove it exactly.
    E64 = E.astype(np.float64)
    for k in range(1, KSEG - 1):
        pad = LROT - sizes[k]
        if pad:
            v = np.linalg.matrix_power(E64, pad) @ np.ones(T)
            Fk = top[:, k, :]
            logZ -= np.log(Fk.T @ v) - np.log(Fk.sum(axis=0))
    return logZ + cnt * MU


def kernel(inputs, transitions, start_transitions, end_transitions, tags, mask):
    inputs = np.ascontiguousarray(np.asarray(inputs), dtype=np.float32)
    transitions = np.asarray(transitions, dtype=np.float32)
    start_transitions = np.asarray(start_transitions, dtype=np.float32)
    end_transitions = np.asarray(end_transitions, dtype=np.float32)
    tags = np.asarray(tags)
    mask_b = np.asarray(mask).astype(bool)

    num = _host_numerator(
        inputs, transitions, start_transitions, end_transitions, tags.astype(np.int64), mask_b
    )

    if not mask_b.all():
        den = _host_denominator(
            inputs.astype(np.float64),
            transitions.astype(np.float64),
            start_transitions.astype(np.float64),
            end_transitions.astype(np.float64),
            mask_b,
        ).astype(np.float32)
        return np.asarray(
            np.float32(num.sum(dtype=np.float32)) - np.float32(den.sum(dtype=np.float32)),
            dtype=np.float32,
        )

    import os

    from concourse import bass_utils

    trace = bool(int(os.environ.get("CRF_TRACE", "0")))
    if trace:
        _ensure_ntff_hook(bass_utils)


    import ml_dtypes

    bf16 = ml_dtypes.bfloat16

    sizes = _segment_sizes()
    bounds = []
    a = 1
    for sz in sizes:
        bounds.append((a, a + sz))
        a += sz
    assert bounds[-1][1] == S

    E = np.exp(transitions).astype(bf16).astype(np.float32)
    blk = np.zeros((128, 128), dtype=bf16)
    blk[:T, :T] = E  # lhsT for fwd: out[j] = sum_i E[i,j] F[i]
    blk[T:, T:] = np.ascontiguousarray(E.T)  # lhsT for bwd: out[j] = sum_i E[j,i] G[i]

    nc = _get_nc()
    in_maps = []
    for c in range(NCORES):
        shard = inputs[c * BPC : (c + 1) * BPC]  # [b, s, j]
        xt = np.ascontiguousarray(shard.transpose(2, 1, 0))  # [j, s, b]
        wem, s0 = _prep_core(xt, start_transitions, end_transitions, sizes, bounds, bf16)
        in_maps.append({"wem": wem, "s0": s0, "blk": blk})

    res = bass_utils.run_bass_kernel_spmd(
        nc, in_maps, core_ids=list(range(NCORES)), trace=trace
    )
    if trace and res.exec_time_ns is not None:
        print(f"HW exec time: {res.exec_time_ns} ns")
        if res.instructions_and_trace is not None:
            print("trace:", res.instructions_and_trace[1])

    logZ = np.concatenate([_stitch(r["fin"], sizes, E) for r in res.results])
    loss = np.float64(num.sum(dtype=np.float64)) - logZ.sum()
    return np.asarray(loss, dtype=np.float32)
